# revision 21
# baseline (speedup 1.0000x reference)
"""TRN2 Bass kernel for the ModalVerlet problem (nn_ModalVerlet_68118181314859).

Math
----
Reference step (per batch b, mode m), k = 1/fs, d = 1 + k*sigma:
    p_dot  = -2*sigma*p - om2*q + g2*tanh(q) + fe0*Phi_e
    p_half = p + (k/2)*p_dot
    q1     = q + k*p_half
    p1     = (p_half + (k/2)*(-om2*q1 + g2*tanh(q1) + fe1*Phi_e)) / d

Eliminating p (p_j = A*ph_{j-1} - beta_h*q_j + eps_h*t_j + fe_j*phi_h) gives a
3-state recurrence in (t=tanh(q), g, bias):
    ph_j     = D1*ph_{j-1} + D2*q_j + D3*t_j + H_j,   H_j = phiH*fe[j]
    q_{j+1}  = q_j + k*ph_j = lam3*t_j + bias_j
    t_{j+1}  = tanh(lam3*t_j + bias_j)      <- single ACT op, the critical cycle
    with g_j = D2*q_j + D1*ph_{j-1} + H_j,  bias_j = k*g_j + q_j,  lam3 = k*D3.

Per-step engine ops (all [128,1] per-partition FMAs via tensor_scalar):
    ACT: t' = tanh(lam3*t + bias)
    DVE: q' = lam3*t + bias;  ph = D3*t + g;  c1 = D1*ph + H';  g' = D2*q' + c1;
         bias' = k*g' + q'
Outputs p_{j+1} = A*ph_j - beta_h*q_{j+1} + eps_h*t_{j+1} + phi_h*fe[j+1] are
reconstructed chunk-wide on GPSIMD (no cancellation); w = Phi_o . q on host.

Sharding: pure data parallel, batch pairs -> 8 cores; partition p = 64*(b%2)+m.
"""
import sys
import numpy as np

if "/opt/trn_rl_repo" not in sys.path:
    sys.path.insert(0, "/opt/trn_rl_repo")

import concourse.bacc as bacc
import concourse.bass as bass
import concourse.mybir as mybir
import concourse.tile as tile
from concourse.bass_utils import run_bass_kernel_spmd

AF = mybir.ActivationFunctionType
ALU = mybir.AluOpType
F32 = mybir.dt.float32

B, M, T = 16, 64, 48000
NCORES = 8
C = 512                  # steps per For_i body
NCH = 94                 # chunks: device covers steps j = 1 .. NCH*C
_CACHE = {}


def tpad_of(nsteps):
    return 2 + nsteps + 4


def build_program(chunk=C, nch=NCH, ncores=NCORES, nch_data=None, reps=1,
                  variant="full"):
    """Build the SPMD Bass program (one NeuronCore's view).

    nch_data sizes the DRAM tensors (defaults to nch); pass nch < nch_data to
    build a timing probe that executes fewer chunks over identically-sized
    I/O so wall-clock differences isolate device time.
    reps > 1 wraps the whole chunk loop in an outer repeat loop (timing only;
    outputs are those of the last rep, which continues from stale state).
    """
    if nch_data is None:
        nch_data = nch
    nsteps = chunk * nch_data
    tpad = tpad_of(nsteps)
    nc = bacc.Bacc("TRN2", target_bir_lowering=False, debug=False,
                   num_devices=ncores)

    coef_in = nc.dram_tensor("coef", [128, 8], F32, kind="ExternalInput")
    state_in = nc.dram_tensor("state0", [128, 4], F32, kind="ExternalInput")
    hforce_in = nc.dram_tensor("hforce", [128, nsteps], F32, kind="ExternalInput")
    fp1_in = nc.dram_tensor("fp1", [128, nsteps], F32, kind="ExternalInput")
    qout = nc.dram_tensor("qout", [128, tpad], F32, kind="ExternalOutput")
    pout = nc.dram_tensor("pout", [128, tpad], F32, kind="ExternalOutput")

    with tile.TileContext(nc) as tc:
        with tc.tile_pool(name="consts", bufs=1) as cpool, \
             tc.tile_pool(name="state", bufs=1) as spool, \
             tc.tile_pool(name="force", bufs=1) as fpool, \
             tc.tile_pool(name="psumw", bufs=1, space="PSUM") as wpool_psum, \
             tc.tile_pool(name="work", bufs=1) as wpool:

            coef = cpool.tile([128, 8], F32)
            nc.sync.dma_start(coef[:], coef_in[:])
            D1 = coef[:, 0:1]
            D2 = coef[:, 1:2]
            D3 = coef[:, 2:3]
            lam3 = coef[:, 3:4]
            A_ = coef[:, 4:5]
            nbeta_h = coef[:, 5:6]   # -beta_h
            eps_h = coef[:, 6:7]
            kcol = coef[:, 7:8]      # k

            # persistent cross-chunk state: cols = t, g, bias (col 3 unused)
            st = spool.tile([128, 4], F32)
            nc.sync.dma_start(st[:], state_in[:])

            import contextlib
            rep_ctx = (tc.For_i(0, reps, 1) if reps > 1
                       else contextlib.nullcontext())
            with rep_ctx, \
                 tc.For_i(0, nch, 1,
                          hint_engines=(mybir.EngineType.Activation,
                                        mybir.EngineType.DVE,
                                        mybir.EngineType.Pool,
                                        mybir.EngineType.SP)) as ci:
                hch = fpool.tile([128, chunk], F32, tag="hch")
                fch = fpool.tile([128, chunk], F32, tag="fch")
                nc.sync.dma_start(hch[:], hforce_in[:, bass.ds(ci * chunk, chunk)])
                nc.sync.dma_start(fch[:], fp1_in[:, bass.ds(ci * chunk, chunk)])

                tch = wpool.tile([128, chunk], F32, tag="tch")
                qch = wpool.tile([128, chunk], F32, tag="qch")
                phch = wpool.tile([128, chunk], F32, tag="phch")
                bch = wpool.tile([128, chunk], F32, tag="bch")
                gch = wpool.tile([128, chunk], F32, tag="gch")
                c1t = wpool.tile([128, chunk], F32, tag="c1t")
                pch = wpool.tile([128, chunk], F32, tag="pch")
                x1 = wpool.tile([128, chunk], F32, tag="x1")
                x2 = wpool.tile([128, chunk], F32, tag="x2")

                do_act = variant in ("full", "actonly", "act1dve",
                                     "actimm", "actpsum")
                ndve = {"full": 5, "actonly": 0, "act1dve": 1,
                        "dveonly": 5, "actimm": 0, "actpsum": 0}[variant]
                if variant == "actpsum":
                    tpsum = wpool_psum.tile([128, chunk], F32, tag="tps")
                for i in range(chunk):
                    t_prev = st[:, 0:1] if i == 0 else (
                        fch[:, i - 1:i] if variant == "dveonly"
                        else tch[:, i - 1:i])
                    g_prev = st[:, 1:2] if i == 0 else gch[:, i - 1:i]
                    b_prev = (st[:, 2:3] if i == 0 else bch[:, i - 1:i]) \
                        if ndve >= 5 else hch[:, i:i + 1]

                    # ACT critical op: t_{j+1}
                    if variant == "actimm":
                        nc.scalar.activation(
                            tch[:, i:i + 1], t_prev, AF.Tanh,
                            bias=0.001, scale=0.9,
                        )
                    elif variant == "actpsum":
                        tp_prev = st[:, 0:1] if i == 0 else tpsum[:, i - 1:i]
                        nc.scalar.activation(
                            tpsum[:, i:i + 1], tp_prev, AF.Tanh,
                            bias=0.001, scale=0.9,
                        )
                        if i == chunk - 1:
                            nc.scalar.activation(
                                tch[:, i:i + 1], tpsum[:, i:i + 1], AF.Tanh,
                                bias=0.001, scale=0.9)
                    elif do_act:
                        nc.scalar.activation(
                            tch[:, i:i + 1], t_prev, AF.Tanh,
                            bias=b_prev, scale=lam3,
                        )
                    # DVE ops
                    if ndve >= 1:
                        nc.vector.tensor_scalar(
                            qch[:, i:i + 1], t_prev, lam3, b_prev,
                            ALU.mult, ALU.add)
                    if ndve >= 5:
                        nc.vector.tensor_scalar(
                            phch[:, i:i + 1], t_prev, D3, g_prev,
                            ALU.mult, ALU.add)
                        nc.vector.tensor_scalar(
                            c1t[:, i:i + 1], phch[:, i:i + 1], D1,
                            hch[:, i:i + 1], ALU.mult, ALU.add)
                        nc.vector.tensor_scalar(
                            gch[:, i:i + 1], qch[:, i:i + 1], D2,
                            c1t[:, i:i + 1], ALU.mult, ALU.add)
                        nc.vector.tensor_scalar(
                            bch[:, i:i + 1], gch[:, i:i + 1], kcol,
                            qch[:, i:i + 1], ALU.mult, ALU.add)

                # carry state to next chunk
                if do_act:
                    nc.vector.tensor_copy(st[:, 0:1], tch[:, chunk - 1:chunk])
                if ndve >= 5:
                    nc.vector.tensor_copy(st[:, 1:2], gch[:, chunk - 1:chunk])
                    nc.vector.tensor_copy(st[:, 2:3], bch[:, chunk - 1:chunk])

                if variant == "full":
                    # wide reconstruction of p_{j+1} on GPSIMD
                    nc.gpsimd.tensor_scalar(x1[:], phch[:], A_, None, ALU.mult)
                    nc.gpsimd.tensor_scalar(x2[:], qch[:], nbeta_h, None, ALU.mult)
                    nc.gpsimd.tensor_tensor(x1[:], x1[:], x2[:], ALU.add)
                    nc.gpsimd.tensor_scalar(x2[:], tch[:], eps_h, None, ALU.mult)
                    nc.gpsimd.tensor_tensor(x2[:], x2[:], fch[:], ALU.add)
                    nc.gpsimd.tensor_tensor(pch[:], x1[:], x2[:], ALU.add)

                # DMA out: chunk covers global output columns [2+ci*C, 2+ci*C+C)
                src_q = tch if ndve == 0 else qch
                nc.sync.dma_start(qout[:, bass.ds(2 + ci * chunk, chunk)], src_q[:])
                src_p = pch if variant == "full" else src_q
                nc.sync.dma_start(pout[:, bass.ds(2 + ci * chunk, chunk)], src_p[:])

    nc.compile()
    return nc


def host_precompute(fs, y0, omega, sigma, gamma, Phi_e, fe_points,
                    chunk=C, nch=NCH, ncores=NCORES):
    """Per-core device inputs + host-computed first columns."""
    f32 = np.float32
    nsteps = chunk * nch
    nb = y0.shape[0]
    k = f32(1.0 / fs)
    y0 = y0.astype(f32)
    omega = omega.astype(f32)
    sigma = sigma.astype(f32)
    gamma = gamma.astype(f32)
    Phi_e = Phi_e.astype(f32)
    fe = fe_points.astype(f32)

    d = (1.0 + k * sigma).astype(f32)
    om2 = (omega ** 2).astype(f32)
    g2 = (gamma ** 2)[:, None].astype(f32)
    alpha = (1.0 - k * sigma).astype(f32)
    beta = ((k / 2) * om2).astype(f32)
    eps = ((k / 2) * g2 * np.ones_like(om2)).astype(f32)
    phi = ((k / 2) * Phi_e).astype(f32)
    A = (1.0 / d).astype(f32)
    beta_h = (beta / d).astype(f32)
    eps_h = (eps / d).astype(f32)
    phi_h = (phi / d).astype(f32)

    D1 = (alpha * A).astype(f32)
    D2 = (-(alpha * beta_h + beta)).astype(f32)
    D3 = (alpha * eps_h + eps).astype(f32)
    phiH = (alpha * phi_h + phi).astype(f32)
    lam3 = (k * D3).astype(f32)

    q0 = y0[:, :M]
    p0 = y0[:, M:]
    t0 = np.tanh(q0).astype(f32)

    # host prologue: step j=0
    ph0 = (alpha * p0 - beta * q0 + eps * t0 + fe[:, 0][:, None] * phi).astype(f32)
    q1 = (q0 + k * ph0).astype(f32)
    t1 = np.tanh(q1).astype(f32)
    p1 = (A * ph0 - beta_h * q1 + eps_h * t1 + fe[:, 1][:, None] * phi_h).astype(f32)

    fe_pad = np.zeros((nb, 2 + nsteps + 4), f32)
    ncols = min(fe.shape[1], fe_pad.shape[1])
    fe_pad[:, :ncols] = fe[:, :ncols]
    H1 = (phiH * fe_pad[:, 1][:, None]).astype(f32)
    c1_1 = (D1 * ph0 + H1).astype(f32)
    g1 = (D2 * q1 + c1_1).astype(f32)
    bias1 = (k * g1 + q1).astype(f32)

    in_maps = []
    for c in range(ncores):
        bsl = slice(2 * c, 2 * c + 2)

        def pack2(x):  # (2, M) -> (128,) partition layout
            return np.ascontiguousarray(x[bsl]).reshape(128)

        coef_np = np.zeros((128, 8), f32)
        coef_np[:, 0] = pack2(D1)
        coef_np[:, 1] = pack2(D2)
        coef_np[:, 2] = pack2(D3)
        coef_np[:, 3] = pack2(lam3)
        coef_np[:, 4] = pack2(A)
        coef_np[:, 5] = pack2(-beta_h)
        coef_np[:, 6] = pack2(eps_h)
        coef_np[:, 7] = k

        state_np = np.zeros((128, 4), f32)
        state_np[:, 0] = pack2(t1)
        state_np[:, 1] = pack2(g1)
        state_np[:, 2] = pack2(bias1)

        fe2 = fe_pad[bsl, 2:2 + nsteps]                      # (2, nsteps)
        fe128 = np.repeat(fe2, M, axis=0)                    # (128, nsteps)
        h_np = (pack2(phiH)[:, None] * fe128).astype(f32)
        f_np = (pack2(phi_h)[:, None] * fe128).astype(f32)

        in_maps.append({
            "coef": coef_np,
            "state0": state_np,
            "hforce": np.ascontiguousarray(h_np),
            "fp1": np.ascontiguousarray(f_np),
        })

    host_cols = dict(q0=q0, q1=q1, p0=p0, p1=p1)
    return in_maps, host_cols


def assemble(results, host_cols, Phi_o, t_out=T, ncores=NCORES):
    y = np.empty((B, 2 * M, t_out), np.float32)
    for c in range(ncores):
        q = results[c]["qout"]   # (128, tpad)
        p = results[c]["pout"]
        for sub in range(2):
            b = 2 * c + sub
            rows = slice(64 * sub, 64 * sub + 64)
            y[b, :M, :] = q[rows, :t_out]
            y[b, M:, :] = p[rows, :t_out]
    y[:, :M, 0] = host_cols["q0"]
    y[:, :M, 1] = host_cols["q1"]
    y[:, M:, 0] = host_cols["p0"]
    y[:, M:, 1] = host_cols["p1"]
    w = np.einsum("bm,bmt->bt", Phi_o.astype(np.float32),
                  y[:, :M, :]).astype(np.float32)
    return y, w


def kernel(fs, num_samples, y0, omega, sigma, gamma, Phi_e, Phi_o, fe_points):
    fs = int(fs)
    num_samples = int(num_samples)
    y0 = np.asarray(y0)
    omega = np.asarray(omega)
    sigma = np.asarray(sigma)
    gamma = np.asarray(gamma)
    Phi_e = np.asarray(Phi_e)
    Phi_o = np.asarray(Phi_o)
    fe_points = np.asarray(fe_points)
    assert num_samples == T and y0.shape == (B, 2 * M)

    in_maps, host_cols = host_precompute(
        fs, y0, omega, sigma, gamma, Phi_e, fe_points)

    if "nc" not in _CACHE:
        _CACHE["nc"] = build_program()
    nc = _CACHE["nc"]

    res = run_bass_kernel_spmd(nc, in_maps, list(range(NCORES)))
    y, w = assemble(res.results, host_cols, Phi_o)
    return (y, w)
